# revision 4
# baseline (speedup 1.0000x reference)
"""Trainium2 Bass kernel for nn_Interactor (attention-augmented LSTM).

B=64, T=512, L=48, DV=DS=H=512; 8 NeuronCores, data-parallel over batch
(8 rows/core), weights replicated. Feature dims on SBUF partitions, the 8
local batch rows on free dims; h state stored as 2h in fp16.

Design highlights (measured 7.56 ms vs 42.3 ms fp32 baseline):
- fp16 weights/streams/elementwise; fp32 PSUM + LSTM cell state
  (rel err ~4.4e-4 vs the 2e-2 gate).
- x_t projections (PV = W_V x, GV = W_ihV x + biases) precomputed with
  N=512 matmuls into partition-major DRAM ([128, T, m, b]) so recurring
  stream DMAs are fully contiguous (16-byte-run layouts previously caused
  a ~30k-descriptor storm per load on the SP engine).
- sigmoid via tanh (sigmoid(x) = 0.5 + 0.5 tanh(x/2)) so every Act op uses
  the exp_and_others table: zero activation-table reloads. Gate order
  permuted to (i,f,o,g) and g-rows pre-doubled so ONE Act call
  tanh(0.5*gates) produces all four gate activations.
- Per-kc pipelined attention front (rvt -> e-add -> tanh -> beta) to
  shorten the serial chain; softmax denominator reduced on DVE right after
  exp (no PE round-trips); exp(beta) strips transposed onto partitions by
  8 tiny PE transposes; h_ts computed on the PE as per-b K=48 matmuls
  against resident h_s (L-on-partition layout); 1/denom applied after via
  a K=1 ones-matmul broadcast.
- Gates computed late in ONE psum with per-m contiguous runs
  [Whh x4; WihS x4] (PSUM accumulation groups must be contiguous: any
  interleaved start=True matmuls corrupt open groups), then a single DVE
  add folds the GV stream.
"""

import numpy as np

import concourse.bass as bass
import concourse.mybir as mybir
import concourse.tile as tile
from concourse import bacc
from concourse.bass_utils import run_bass_kernel_spmd

F32 = mybir.dt.float32
F16 = mybir.dt.float16
AF = mybir.ActivationFunctionType
ALU = mybir.AluOpType
AX = mybir.AxisListType

B, T_FULL, L = 64, 512, 48
DV, DS, H = 512, 512, 512
G4 = 4 * H
NCORES = 8
BLOC = B // NCORES  # 8
BL = BLOC * L       # 384
KH = H // 128       # 4
KM = G4 // 128      # 16
L64 = 64            # padded L for block-diag hs layout


def build_nc(T=T_FULL, Tc=16, debug=False, dbg_t=0):
    assert T % Tc == 0
    nc = bacc.Bacc()

    # ---- DRAM I/O ----
    hvT = nc.declare_dram_parameter("hvT", [DV, T * BLOC], F16, isOutput=False)
    hsT = nc.declare_dram_parameter("hsT", [DS, BL], F16, isOutput=False)
    hs_bT = nc.declare_dram_parameter("hs_bT", [L, BLOC, DS], F16, isOutput=False)
    WS_T = nc.declare_dram_parameter("WS_T", [DS, H], F16, isOutput=False)
    WV_T = nc.declare_dram_parameter("WV_T", [DV, H], F16, isOutput=False)
    WihV_T = nc.declare_dram_parameter("WihV_T", [DV, G4], F16, isOutput=False)
    WihS_T = nc.declare_dram_parameter("WihS_T", [DS, G4], F16, isOutput=False)
    Whh_T = nc.declare_dram_parameter("Whh_T", [H, G4], F16, isOutput=False)
    WR_T = nc.declare_dram_parameter("WR_T", [H, H], F16, isOutput=False)
    wvec = nc.declare_dram_parameter("wvec", [H, 1], F16, isOutput=False)
    biasRSV = nc.declare_dram_parameter("biasRSV", [128, KH], F32, isOutput=False)
    biasIH = nc.declare_dram_parameter("biasIH", [128, KM], F32, isOutput=False)
    bw = nc.declare_dram_parameter("bw", [1, 1], F32, isOutput=False)
    out_c = nc.declare_dram_parameter("out_c", [128, T, KH, BLOC], F32, isOutput=True)

    if debug:
        dbg_rvt = nc.dram_tensor("dbg_rvt", [128, KH * BLOC], F16, kind="ExternalOutput")
        dbg_e = nc.dram_tensor("dbg_e", [128, KH, BL], F16, kind="ExternalOutput")
        dbg_expb = nc.dram_tensor("dbg_expb", [1, BL], F32, kind="ExternalOutput")
        dbg_rhsbd = nc.dram_tensor("dbg_rhsbd", [L, BLOC], F16, kind="ExternalOutput")
        dbg_hts = nc.dram_tensor("dbg_hts", [128, KH * BLOC], F16, kind="ExternalOutput")
        dbg_gates = nc.dram_tensor("dbg_gates", [128, KM * BLOC], F32, kind="ExternalOutput")
        dbg_hprev = nc.dram_tensor("dbg_hprev", [128, KH * BLOC], F16, kind="ExternalOutput")
        dbg_cprev = nc.dram_tensor("dbg_cprev", [128, KH * BLOC], F32, kind="ExternalOutput")

    # internal DRAM for precomputed x_t projections (fp16)
    GV_d = nc.dram_tensor("GV_d", [128, T, KM, BLOC], F16)
    PV_d = nc.dram_tensor("PV_d", [128, T, KH, BLOC], F16)

    NT = T * BLOC
    NCW = min(512, NT)
    n_nc = NT // NCW

    with tile.TileContext(nc) as tc:
        with (
            tc.tile_pool(name="res", bufs=1) as res,
            tc.tile_pool(name="state", bufs=2) as state,
            tc.tile_pool(name="hout", bufs=2) as houtp,
            tc.tile_pool(name="stream", bufs=2) as stream,
            tc.tile_pool(name="work", bufs=2) as work,
            tc.tile_pool(name="ppersist", bufs=1, space="PSUM") as ppersist,
        ):
            # ---------- resident loads ----------
            wr_sb = res.tile([128, KH, H], F16, tag="wr")
            nc.sync.dma_start(out=wr_sb, in_=WR_T.rearrange("(k p) m -> p k m", p=128))
            wihs_sb = res.tile([128, KH, G4], F16, tag="wihs")
            nc.sync.dma_start(out=wihs_sb, in_=WihS_T.rearrange("(k p) m -> p k m", p=128))
            whh_sb = res.tile([128, KH, G4], F16, tag="whh")
            nc.sync.dma_start(out=whh_sb, in_=Whh_T.rearrange("(k p) m -> p k m", p=128))
            hs_sb = res.tile([128, KH, BL], F16, tag="hs")
            nc.sync.dma_start(out=hs_sb, in_=hsT.rearrange("(k p) n -> p k n", p=128))
            hsb_sb = res.tile([L, BLOC, DS], F16, tag="hsb")
            nc.sync.dma_start(out=hsb_sb, in_=hs_bT[:, :, :])
            wvec_sb = res.tile([128, KH], F16, tag="wvec")
            nc.sync.dma_start(out=wvec_sb, in_=wvec.rearrange("(k p) o -> p (k o)", p=128))
            brsv_sb = res.tile([128, KH], F32, tag="brsv")
            nc.sync.dma_start(out=brsv_sb, in_=biasRSV[:, :])
            bih_sb = res.tile([128, KM], F32, tag="bih")
            nc.sync.dma_start(out=bih_sb, in_=biasIH[:, :])
            bw_sb = res.tile([1, 1], F32, tag="bw")
            nc.sync.dma_start(out=bw_sb, in_=bw[:, :])
            ones1 = res.tile([1, 128], F16, tag="ones1")
            nc.vector.memset(ones1, 1.0)
            ones48 = res.tile([L, 1], F16, tag="ones48")
            nc.vector.memset(ones48, 1.0)
            ident11 = res.tile([1, 1], F32, tag="ident11")
            nc.vector.memset(ident11, 1.0)
            ps_sb = res.tile([128, KH, BL], F16, tag="ps")


            # ---------- precompute phase ----------
            with (
                tc.tile_pool(name="prew", bufs=1) as prew,
                tc.tile_pool(name="prehv", bufs=4) as prehv,
                tc.tile_pool(name="prestg", bufs=3) as prestg,
                tc.tile_pool(name="prepsum", bufs=4, space="PSUM") as prepsum,
            ):
                ws_sb = prew.tile([128, KH, H], F16, tag="ws")
                nc.sync.dma_start(out=ws_sb, in_=WS_T.rearrange("(k p) m -> p k m", p=128))
                wv_sb = prew.tile([128, KH, H], F16, tag="wv")
                nc.sync.dma_start(out=wv_sb, in_=WV_T.rearrange("(k p) m -> p k m", p=128))
                wihv_sb = prew.tile([128, KH, G4], F16, tag="wihv")
                nc.sync.dma_start(out=wihv_sb, in_=WihV_T.rearrange("(k p) m -> p k m", p=128))

                # PS = W_S @ hsT + biasRSV  (stored fp16)
                for m in range(KH):
                    pps = prepsum.tile([128, 512], F32, tag="pp")
                    for kc in range(KH):
                        nc.tensor.matmul(
                            pps[:, :BL],
                            ws_sb[:, kc, m * 128:(m + 1) * 128],
                            hs_sb[:, kc, :],
                            start=(kc == 0), stop=(kc == KH - 1),
                        )
                    nc.vector.tensor_scalar_add(ps_sb[:, m, :], pps[:, :BL], brsv_sb[:, m:m + 1])

                for ncnk in range(n_nc):
                    nsl = slice(ncnk * NCW, (ncnk + 1) * NCW)
                    hv_t = []
                    for kc in range(KH):
                        t_ = prehv.tile([128, NCW], F16, tag="hv")
                        nc.sync.dma_start(out=t_, in_=hvT[kc * 128:(kc + 1) * 128, nsl])
                        hv_t.append(t_)
                    t0 = ncnk * NCW // BLOC
                    tw = NCW // BLOC
                    stg_pv = prestg.tile([128, tw, KH, BLOC], F16, tag="pvstg")
                    for m in range(KH):
                        ppv = prepsum.tile([128, NCW], F32, tag="pp")
                        for kc in range(KH):
                            nc.tensor.matmul(
                                ppv, wv_sb[:, kc, m * 128:(m + 1) * 128],
                                hv_t[kc], start=(kc == 0), stop=(kc == KH - 1))
                        nc.vector.tensor_copy(
                            stg_pv[:, :, m, :],
                            ppv.rearrange("p (t b) -> p t b", b=BLOC))
                    nc.sync.dma_start(out=PV_d[:, t0:t0 + tw, :, :], in_=stg_pv)
                    stg_gv = prestg.tile([128, tw, KM, BLOC], F16, tag="gvstg")
                    for m in range(KM):
                        pgv = prepsum.tile([128, NCW], F32, tag="pp")
                        for kc in range(KH):
                            nc.tensor.matmul(
                                pgv, wihv_sb[:, kc, m * 128:(m + 1) * 128],
                                hv_t[kc], start=(kc == 0), stop=(kc == KH - 1))
                        pgv3 = pgv.rearrange("p (t b) -> p t b", b=BLOC)
                        if m % 2 == 0:
                            nc.vector.tensor_scalar_add(
                                stg_gv[:, :, m, :], pgv3, bih_sb[:, m:m + 1])
                        else:
                            nc.scalar.activation(
                                stg_gv[:, :, m, :], pgv3, AF.Identity,
                                bias=bih_sb[:, m:m + 1])
                    nc.sync.dma_start(out=GV_d[:, t0:t0 + tw, :, :], in_=stg_gv)

            # ---------- recurrence ----------
            psum = tc.alloc_tile_pool(name="psum", bufs=2, space="PSUM")
            psumg_pool = tc.alloc_tile_pool(name="psumg", bufs=2, space="PSUM")
            czero = state.tile([128, 32], F32, tag="c")
            nc.vector.memset(czero, 0.0)
            hzero = res.tile([128, 32], F16, tag="h0")
            nc.vector.memset(hzero, 0.0)
            c_prev = czero
            h_prev = hzero

            HB = 8
            gv_cur = pv_cur = None
            hbuf = None

            for t in range(T):
                ic = t % Tc
                if ic == 0:
                    gv_cur = stream.tile([128, Tc, KM, BLOC], F16, tag="gv")
                    nc.sync.dma_start(out=gv_cur, in_=GV_d[:, t:t + Tc, :, :])
                    pv_cur = stream.tile([128, Tc, KH, BLOC], F16, tag="pv")
                    nc.sync.dma_start(out=pv_cur, in_=PV_d[:, t:t + Tc, :, :])
                ts_ = t % HB
                if ts_ == 0:
                    hbuf = houtp.tile([128, HB, KH, BLOC], F32, tag="hb")

                # --- PE: proj_R (needs h_prev) ---
                # one packed psum bank: rv [0:32), rec128 [32:40), hts [40:72),
                # beta on partition 0 cols [128:512)
                psumA = psum.tile([128, 512], F32, tag="pa")
                psum_rv = psumA[:, 0:32].rearrange("p (k b) -> p k b", b=BLOC)
                for m in range(KH):
                    for kc in range(KH):
                        nc.tensor.matmul(
                            psum_rv[:, m, :],
                            wr_sb[:, kc, m * 128:(m + 1) * 128],
                            h_prev[:, kc * BLOC:(kc + 1) * BLOC],
                            start=(kc == 0), stop=(kc == KH - 1))
                psum_g = psumg_pool.tile([128, KM, BLOC], F32, tag="g")

                # --- per-kc pipeline: rvt_kc -> e-add_kc -> tanh_kc (beta below) ---
                rvt = work.tile([128, KH * BLOC], F16, tag="rvt")
                e_all = work.tile([128, KH, BL], F16, tag="e")
                for kc in range(KH):
                    nc.vector.tensor_tensor(
                        rvt[:, kc * BLOC:(kc + 1) * BLOC],
                        psum_rv[:, kc, :], pv_cur[:, ic, kc, :], ALU.add)
                    sl = rvt[:, kc * BLOC:(kc + 1) * BLOC]
                    bck = bass.AP(tensor=sl.tensor, offset=sl.offset,
                                  ap=[sl.ap[0], [1, BLOC], [0, L]])
                    nc.vector.tensor_tensor(
                        e_all[:, kc].rearrange("p (b l) -> p b l", l=L),
                        ps_sb[:, kc].rearrange("p (b l) -> p b l", l=L),
                        bck, ALU.add)
                    nc.scalar.activation(e_all[:, kc], e_all[:, kc], AF.Tanh)

                # --- PE: beta (emitted after WhhA; per-kc inputs arrive pipelined) ---
                psum_beta = psumA[0:1, 128:128 + BL]
                for kc in range(KH):
                    nc.tensor.matmul(
                        psum_beta, wvec_sb[:, kc:kc + 1], e_all[:, kc],
                        start=(kc == 0), stop=(kc == KH - 1))
                # --- softmax: exp; denom on DVE (hop-free after exp);
                # transpose strips on PE in parallel ---
                expb = work.tile([1, BL], F32, tag="expb")
                nc.scalar.activation(expb, psum_beta, AF.Exp, bias=bw_sb[:, 0:1])
                denom = work.tile([1, BLOC], F32, tag="denom")
                nc.vector.tensor_reduce(
                    denom, expb.rearrange("p (b l) -> p b l", l=L), AX.X, ALU.add)
                rec = work.tile([1, BLOC], F16, tag="rec")
                with nc.allow_low_precision(reason="softmax 1/denom in fp16 is ample"):
                    nc.vector.reciprocal(rec, denom)
                psum_expT = psumA[0:L, 72:72 + BLOC]
                for c_ in range(BLOC):
                    nc.tensor.transpose(
                        psum_expT[:, c_:c_ + 1],
                        expb[:, L * c_:L * (c_ + 1)],
                        ident11)
                rhs_bd = work.tile([L, BLOC], F16, tag="rhsbd")
                nc.vector.tensor_copy(rhs_bd, psum_expT)
                # --- PE: h_ts matmuls (hs per-b stationary) ---
                psum_hts = psumA[:, 40:72].rearrange("p (k b) -> p k b", b=BLOC)
                for b_ in range(BLOC):
                    for kc in range(KH):
                        nc.tensor.matmul(
                            psum_hts[:, kc, b_:b_ + 1],
                            hsb_sb[:, b_, kc * 128:(kc + 1) * 128],
                            rhs_bd[:, b_:b_ + 1],
                            start=True, stop=True)
                psum_rec = psumA[:, 32:40]
                nc.tensor.matmul(psum_rec, ones1, rec, start=True, stop=True)
                rec128_sb = work.tile([128, BLOC], F16, tag="rec128sb")
                nc.vector.tensor_copy(rec128_sb, psum_rec)
                h_ts = work.tile([128, KH * BLOC], F16, tag="hts")
                rb = bass.AP(tensor=rec128_sb.tensor, offset=rec128_sb.offset,
                             ap=[rec128_sb.ap[0], [0, KH], [1, BLOC]])
                nc.vector.tensor_tensor(
                    h_ts.rearrange("p (k b) -> p k b", b=BLOC),
                    psum_hts, rb, ALU.mult)

                # --- PE: gates block, contiguous per-m runs [Whh x4; WihS x4] ---
                for m in range(KM):
                    for kc in range(KH):
                        nc.tensor.matmul(
                            psum_g[:, m, :],
                            whh_sb[:, kc, m * 128:(m + 1) * 128],
                            h_prev[:, kc * BLOC:(kc + 1) * BLOC],
                            start=(kc == 0), stop=False)
                    for kc in range(KH):
                        nc.tensor.matmul(
                            psum_g[:, m, :],
                            wihs_sb[:, kc, m * 128:(m + 1) * 128],
                            h_ts[:, kc * BLOC:(kc + 1) * BLOC],
                            start=False, stop=(kc == KH - 1))
                gates = work.tile([128, KM * BLOC], F32, tag="gates")
                nc.vector.tensor_tensor(
                    gates.rearrange("p (m b) -> p m b", b=BLOC),
                    psum_g, gv_cur[:, ic], ALU.add)

                # --- LSTM pointwise; cols (i,f,o,g); g-rows pre-doubled so one
                # Act call computes tanh(i/2), tanh(f/2), tanh(o/2), tanh(g) ---
                ts_a = work.tile([128, 128], F32, tag="tsa")
                nc.scalar.activation(ts_a, gates, AF.Tanh, scale=0.5)
                sfc = work.tile([128, 32], F32, tag="sfc")
                nc.vector.scalar_tensor_tensor(
                    sfc, ts_a[:, 32:64], 1.0, c_prev, ALU.add, ALU.mult)
                sig = work.tile([128, 32], F32, tag="sig")
                nc.vector.scalar_tensor_tensor(
                    sig, ts_a[:, 0:32], 1.0, ts_a[:, 96:128], ALU.add, ALU.mult)
                s2c = work.tile([128, 32], F32, tag="s2c")
                nc.vector.tensor_tensor(s2c, sfc, sig, ALU.add)
                c_new = state.tile([128, 32], F32, tag="c")
                nc.vector.tensor_scalar_mul(c_new, s2c, 0.5)
                tc_ = work.tile([128, 32], F32, tag="tc")
                nc.scalar.activation(tc_, s2c, AF.Tanh, scale=0.5)
                # h state kept as 2h (W_R/W_hh pre-halved on host); fp16 direct
                h_new16 = state.tile([128, 32], F16, tag="h16")
                nc.vector.scalar_tensor_tensor(
                    h_new16, ts_a[:, 64:96], 1.0, tc_, ALU.add, ALU.mult)
                nc.vector.tensor_scalar_mul(
                    hbuf[:, ts_].rearrange("p k b -> p (k b)"), h_new16, 0.5)

                if debug and t == dbg_t:
                    nc.sync.dma_start(out=dbg_rvt[:, :], in_=rvt)
                    nc.sync.dma_start(out=dbg_e[:, :, :], in_=e_all)
                    nc.sync.dma_start(out=dbg_expb[:, :], in_=expb)
                    nc.sync.dma_start(out=dbg_rhsbd[:, :], in_=rhs_bd)
                    nc.sync.dma_start(out=dbg_hts[:, :], in_=h_ts)
                    nc.sync.dma_start(out=dbg_gates[:, :], in_=gates)
                    nc.sync.dma_start(out=dbg_hprev[:, :], in_=h_prev)
                    nc.sync.dma_start(out=dbg_cprev[:, :], in_=c_prev)

                c_prev = c_new
                h_prev = h_new16
                if ts_ == HB - 1 or t == T - 1:
                    nb = ts_ + 1
                    t0 = t - nb + 1
                    nc.sync.dma_start(
                        out=out_c[:, t0:t0 + nb, :, :], in_=hbuf[:, :nb])
            psumg_pool.release()
            psum.release()
    nc.finalize()
    return nc


# ---------------- host side ----------------

# gate permutation: torch order (i, f, g, o) -> kernel order (i, f, o, g)
def permute_gates(w):
    """w: [4H, ...] rows in (i,f,g,o) order -> (i,f,o,g) order."""
    i, f, g, o = np.split(w, 4, axis=0)
    return np.concatenate([i, f, o, g], axis=0)


def prep_core_inputs(h_v, h_s, W, T=T_FULL):
    f16 = np.float16
    WS_T = np.ascontiguousarray(W["W_S"].T).astype(f16)
    WV_T = np.ascontiguousarray(W["W_V"].T).astype(f16)
    # gate scaling: g-rows doubled (single tanh(x/2) Act call recovers tanh(g));
    # W_R/W_hh halved because the h state is stored as 2h.
    gsc = np.ones((4 * H, 1), np.float32)
    gsc[3 * H:] = 2.0
    Wih_p = permute_gates(W["W_ih"]) * gsc
    Whh_p = permute_gates(W["W_hh"]) * gsc * 0.5
    bih_p = (permute_gates((W["b_ih"] + W["b_hh"])[:, None]) * gsc)[:, 0]
    WihV_T = np.ascontiguousarray(Wih_p[:, :DV].T).astype(f16)
    WihS_T = np.ascontiguousarray(Wih_p[:, DV:].T).astype(f16)
    Whh_T = np.ascontiguousarray(Whh_p.T).astype(f16)
    WR_T = np.ascontiguousarray(W["W_R"].T).astype(f16) * f16(0.5)
    wvec = np.ascontiguousarray(W["W_w"][0][:, None]).astype(f16)
    biasRSV = np.ascontiguousarray(
        (W["b_S"] + W["b_R"] + W["b_V"]).reshape(KH, 128).T).astype(np.float32)
    biasIH = np.ascontiguousarray(bih_p.reshape(KM, 128).T).astype(np.float32)
    bw = np.ascontiguousarray(W["b_w"].reshape(1, 1)).astype(np.float32)
    maps = []
    for c in range(NCORES):
        bs = slice(c * BLOC, (c + 1) * BLOC)
        hvT = np.ascontiguousarray(
            h_v[bs, :T].transpose(2, 1, 0).reshape(DV, T * BLOC)).astype(f16)
        hsT = np.ascontiguousarray(
            h_s[bs].transpose(2, 0, 1).reshape(DS, BLOC * L)).astype(f16)
        # per-b L-on-partitions layout: hs_bT[l, b, d] = h_s[b, l, d]
        hs_b = np.ascontiguousarray(
            h_s[bs].transpose(1, 0, 2)).astype(f16)  # [L, BLOC, DS]
        maps.append({
            "hvT": hvT, "hsT": hsT, "hs_bT": hs_b, "WS_T": WS_T, "WV_T": WV_T,
            "WihV_T": WihV_T, "WihS_T": WihS_T, "Whh_T": Whh_T, "WR_T": WR_T,
            "wvec": wvec, "biasRSV": biasRSV, "biasIH": biasIH, "bw": bw,
        })
    return maps


_NC_CACHE = {}


def kernel(**inputs):
    h_v = np.asarray(inputs["h_v"], dtype=np.float32)
    h_s = np.asarray(inputs["h_s"], dtype=np.float32)
    W = {k: np.asarray(v, dtype=np.float32) for k, v in inputs.items()}
    key = "full"
    if key not in _NC_CACHE:
        _NC_CACHE[key] = build_nc(T=T_FULL, Tc=16)
    nc = _NC_CACHE[key]
    maps = prep_core_inputs(h_v, h_s, W, T=T_FULL)
    res = run_bass_kernel_spmd(nc, maps, list(range(NCORES)))
    outs = []
    for c in range(NCORES):
        arr = res.results[c]["out_c"]  # [128, T, KH, BLOC]
        outs.append(np.ascontiguousarray(
            arr.transpose(3, 1, 2, 0)).reshape(BLOC, T_FULL, H))
    return np.concatenate(outs, axis=0).astype(np.float32)


if __name__ == "__main__":
    nc = build_nc(T=8, Tc=4)
    print("built ok")
